# revision 13
# baseline (speedup 1.0000x reference)
"""CGCNN Interactions (NNConv-style message passing) on 8 TRN2 NeuronCores.

Strategy (edge-parallel, sharded by destination-node range):
  - core m owns nodes [m*1250, (m+1)*1250) and ALL edges whose dst falls there.
  - per-edge weight matrices are never materialized: per conv iteration,
    msg^T = W2'^T @ U^T where U^T[(c,i),e] = z[e,c]*x[e,i] is built on the fly
    (PE broadcast-matmul for z replication, DVE multiply, PE main matmul).
  - mean-scatter is a one-hot matmul into per-node-block PSUM accumulators;
    1/cnt is folded into the gathered x rows (msg is linear in x).
  - node features are exchanged via AllGather (bf16) between iterations;
    gather of out[src] is a single indirect DMA per iteration.

kernel(**inputs) takes FULL inputs, shards on host, runs one NEFF on cores
0..7 via run_bass_kernel_spmd, and reassembles the full [10000, 64] output.
"""

import math
from contextlib import ExitStack

import numpy as np
import ml_dtypes

import concourse.bass as bass
import concourse.bacc as bacc
import concourse.tile as tile
import concourse.mybir as mybir
from concourse.bass import IndirectOffsetOnAxis
from concourse.bass_utils import run_bass_kernel_spmd
from concourse.masks import make_identity

BF16 = mybir.dt.bfloat16
F32 = mybir.dt.float32
I32 = mybir.dt.int32
NPBF16 = ml_dtypes.bfloat16

# problem constants
N = 10000
E = 50000
HC = 64
NF = 64
NG = 5
NCORES = 8
NPC = N // NCORES          # 1250 nodes owned per core
NPAD = 1280                # padded to 10 x 128 rows
BLK = 512                  # node block (scatter matmul free dim)
NBLK = math.ceil(NPC / BLK)  # 3
N_CONV = 2

ALL_CORES = list(range(NCORES))


# ---------------------------------------------------------------- host prep

def _prep(inputs):
    src = np.asarray(inputs["edge_index"])[0].astype(np.int64)
    dst = np.asarray(inputs["edge_index"])[1].astype(np.int64)
    ea = np.asarray(inputs["edge_attr"], dtype=np.float32)

    core = dst // NPC
    dstloc = dst - core * NPC
    blk = dstloc // BLK

    cnt = np.bincount(dst, minlength=N).astype(np.float32)
    cntc = np.maximum(cnt, 1.0)
    invc_e = (1.0 / cntc)[dst].astype(np.float32)
    srcrow = ((src // NPC) * NPAD + (src % NPC)).astype(np.int32)

    counts = np.zeros((NCORES, NBLK), np.int64)
    np.add.at(counts, (core, blk), 1)
    Bb = (np.ceil(counts.max(axis=0) / 128).astype(np.int64)) * 128
    epad = int(Bb.sum())
    tail = (-epad) % 512
    Bb[-1] += tail
    epad += tail
    nchunk = epad // 128
    ntile = epad // 512
    blk_base = np.concatenate([[0], np.cumsum(Bb)])[:NBLK].astype(np.int64)

    blk_of_chunk = np.repeat(np.arange(NBLK), Bb // 128)
    chunk_first = np.zeros(nchunk, bool)
    chunk_last = np.zeros(nchunk, bool)
    for b in range(NBLK):
        c0 = int(blk_base[b]) // 128
        c1 = c0 + int(Bb[b]) // 128
        chunk_first[c0] = True
        chunk_last[c1 - 1] = True

    srcrow_a = np.zeros((NCORES, epad), np.int32)
    dstloc_a = np.full((NCORES, epad), -1, np.int64)
    invc_a = np.zeros((NCORES, epad), np.float32)
    ea_a = np.zeros((NCORES, epad, NG), np.float32)
    for m in range(NCORES):
        for b in range(NBLK):
            idx = np.nonzero((core == m) & (blk == b))[0]
            o = int(blk_base[b])
            n = len(idx)
            srcrow_a[m, o:o + n] = srcrow[idx]
            dstloc_a[m, o:o + n] = dstloc[idx] - b * BLK
            invc_a[m, o:o + n] = invc_e[idx]
            ea_a[m, o:o + n] = ea[idx]

    # one-hot scatter matrices with 1/cnt folded in:
    # s_onehot[m, p, ch*BLK + n] = invcnt if dstloc(edge ch*128+p) == n else 0
    s_onehot = np.zeros((NCORES, 128, nchunk * BLK), NPBF16)
    e_idx = np.arange(epad)
    p_of_e = e_idx % 128
    ch_of_e = e_idx // 128
    for m in range(NCORES):
        real = dstloc_a[m] >= 0
        s_onehot[m, p_of_e[real],
                 ch_of_e[real] * BLK + dstloc_a[m][real]] = invc_a[m][real]

    def dev128(a):  # [.., epad] -> [.., 128, nchunk] device layout (p = e%128)
        return np.ascontiguousarray(
            a.reshape(a.shape[:-1] + (nchunk, 128)).swapaxes(-1, -2))

    h = np.asarray(inputs["h"], np.float32)
    hT_own = np.zeros((NCORES, HC, NPAD), np.float32)
    for m in range(NCORES):
        hT_own[m, :, :NPC] = h[m * NPC:(m + 1) * NPC].T

    # weights
    w = {}
    w["lin0_w"] = np.asarray(inputs["lin0_w"], np.float32)             # [64,64] lhsT
    w["lin0_b"] = np.asarray(inputs["lin0_b"], np.float32)[:, None]    # [64,1]
    w["short_w"] = np.asarray(inputs["short_w"], np.float32)           # [5,3]
    w["short_b"] = np.asarray(inputs["short_b"], np.float32)[:, None]  # [3,1]
    w["nn1_w"] = np.asarray(inputs["nn1_w"], np.float32)               # [3,64]
    w["nn1_b"] = np.asarray(inputs["nn1_b"], np.float32)[:, None]      # [64,1]
    w["root_w"] = np.asarray(inputs["root_w"], np.float32)             # [64,64]
    w["conv_b"] = np.asarray(inputs["conv_b"], np.float32)[:, None]    # [64,1]
    w2 = np.asarray(inputs["nn2_w"], np.float32).reshape(HC * HC, NF)  # [(c,i),o]
    w["w2p"] = np.ascontiguousarray(
        w2.reshape(32, 128, NF).transpose(1, 0, 2).reshape(128, 32 * NF)
    ).astype(NPBF16)                                                   # [128, 32*64]
    w["b2"] = np.asarray(inputs["nn2_b"], np.float32).reshape(HC, NF).astype(NPBF16)

    sel = np.zeros((64, 32 * 128), np.float32)
    for kc in range(32):
        for p in range(128):
            sel[2 * kc + p // 64, kc * 128 + p] = 1.0
    w["sel_all"] = sel.astype(NPBF16)

    meta = dict(epad=epad, nchunk=nchunk, ntile=ntile,
                blk_of_chunk=blk_of_chunk, chunk_first=chunk_first,
                chunk_last=chunk_last)
    per_core = dict(
        srcrow=dev128(srcrow_a),      # [8,128,nchunk] i32
        s_onehot=s_onehot,            # [8,128,nchunk*BLK] bf16
        eaT=np.ascontiguousarray(ea_a.swapaxes(1, 2)),  # [8,5,epad] f32
        hT_own=hT_own,                # [8,64,1280] f32
    )
    return meta, per_core, w


# ---------------------------------------------------------------- program

def _build(meta):
    epad = meta["epad"]
    nchunk = meta["nchunk"]
    ntile = meta["ntile"]
    blk_of_chunk = meta["blk_of_chunk"]
    chunk_first = meta["chunk_first"]
    chunk_last = meta["chunk_last"]

    nc = bacc.Bacc("TRN2", target_bir_lowering=False, debug=False,
                   enable_asserts=False, num_devices=NCORES)

    t_in = {}
    t_in["srcrow"] = nc.dram_tensor("srcrow", [128, nchunk], I32, kind="ExternalInput")
    t_in["s_onehot"] = nc.dram_tensor("s_onehot", [128, nchunk * BLK], BF16,
                                      kind="ExternalInput")
    t_in["eaT"] = nc.dram_tensor("eaT", [NG, epad], F32, kind="ExternalInput")
    t_in["hT_own"] = nc.dram_tensor("hT_own", [HC, NPAD], F32, kind="ExternalInput")
    t_in["lin0_w"] = nc.dram_tensor("lin0_w", [HC, NF], F32, kind="ExternalInput")
    t_in["lin0_b"] = nc.dram_tensor("lin0_b", [NF, 1], F32, kind="ExternalInput")
    t_in["short_w"] = nc.dram_tensor("short_w", [NG, 3], F32, kind="ExternalInput")
    t_in["short_b"] = nc.dram_tensor("short_b", [3, 1], F32, kind="ExternalInput")
    t_in["nn1_w"] = nc.dram_tensor("nn1_w", [3, HC], F32, kind="ExternalInput")
    t_in["nn1_b"] = nc.dram_tensor("nn1_b", [HC, 1], F32, kind="ExternalInput")
    t_in["root_w"] = nc.dram_tensor("root_w", [NF, NF], F32, kind="ExternalInput")
    t_in["conv_b"] = nc.dram_tensor("conv_b", [NF, 1], F32, kind="ExternalInput")
    t_in["w2p"] = nc.dram_tensor("w2p", [128, 32 * NF], BF16, kind="ExternalInput")
    t_in["b2"] = nc.dram_tensor("b2", [HC, NF], BF16, kind="ExternalInput")
    t_in["sel_all"] = nc.dram_tensor("sel_all", [64, 32 * 128], BF16, kind="ExternalInput")

    out_own = nc.dram_tensor("out_own", [NPAD, NF], F32, kind="ExternalOutput")
    own_rows = nc.dram_tensor("own_rows", [NPAD, NF], BF16)
    outbuf = nc.dram_tensor("outbuf", [NCORES * NPAD, NF], BF16, addr_space="Shared")

    with tile.TileContext(nc) as tc, ExitStack() as ctx:
        cp = ctx.enter_context(tc.tile_pool(name="const", bufs=1))
        wp = ctx.enter_context(tc.tile_pool(name="work", bufs=3))
        pb = ctx.enter_context(tc.tile_pool(name="pb", bufs=2, space="PSUM"))
        pxt = ctx.enter_context(tc.tile_pool(name="pxt", bufs=1, space="PSUM"))
        pmsg = ctx.enter_context(tc.tile_pool(name="pmsg", bufs=1, space="PSUM"))
        pmr = ctx.enter_context(tc.tile_pool(name="pmr", bufs=1, space="PSUM"))
        pagg = ctx.enter_context(tc.tile_pool(name="pagg", bufs=1, space="PSUM"))

        # ---- constants
        def cload(name, shape, dtype):
            t = cp.tile(shape, dtype, tag=name)
            nc.sync.dma_start(t[:], t_in[name].ap())
            return t

        srcrow_s = cload("srcrow", [128, nchunk], I32)
        sone_s = cload("s_onehot", [128, nchunk * BLK], BF16)
        eaT_s = cload("eaT", [NG, epad], F32)
        hT_s = cload("hT_own", [HC, NPAD], F32)
        lin0w_s = cload("lin0_w", [HC, NF], F32)
        lin0b_s = cload("lin0_b", [NF, 1], F32)
        shortw_s = cload("short_w", [NG, 3], F32)
        shortb_s = cload("short_b", [3, 1], F32)
        nn1w_s = cload("nn1_w", [3, HC], F32)
        nn1b_s = cload("nn1_b", [HC, 1], F32)
        rootw_s = cload("root_w", [NF, NF], F32)
        convb_s = cload("conv_b", [NF, 1], F32)
        w2p_s = cload("w2p", [128, 32 * NF], BF16)
        b2_s = cload("b2", [HC, NF], BF16)
        sel_s = cload("sel_all", [64, 32 * 128], BF16)

        ident_bf = cp.tile([128, 128], BF16, tag="identb")
        make_identity(nc, ident_bf[:])
        ident_f = cp.tile([128, 128], F32, tag="identf")
        make_identity(nc, ident_f[:])

        zT_s = cp.tile([64, epad], BF16, tag="zT")
        aggsb = cp.tile([64, NBLK * BLK], F32, tag="aggsb")
        outT = [cp.tile([64, NPAD], F32, tag=f"outT{i}", name=f"outT{i}")
                for i in range(2)]

        # ---- z precompute: z^T = relu(nn1^T relu(short^T eaT + sb) + nb)
        for t in range(ntile):
            sl = slice(512 * t, 512 * (t + 1))
            p_ea = pmsg.tile([3, 512], F32, tag="msg")
            nc.tensor.matmul(p_ea[:], lhsT=shortw_s[:], rhs=eaT_s[:, sl],
                             start=True, stop=True, skip_group_check=True)
            ea_r = wp.tile([3, 512], F32, tag="ear")
            nc.scalar.activation(ea_r[:], p_ea[:],
                                 mybir.ActivationFunctionType.Relu,
                                 bias=shortb_s[:])
            p_z = pmsg.tile([64, 512], F32, tag="msg")
            nc.tensor.matmul(p_z[:], lhsT=nn1w_s[:], rhs=ea_r[:],
                             start=True, stop=True, skip_group_check=True)
            nc.scalar.activation(zT_s[:, sl], p_z[:],
                                 mybir.ActivationFunctionType.Relu,
                                 bias=nn1b_s[:])

        # ---- helpers
        col_groups = [(slice(0, 512), 512), (slice(512, 1024), 512),
                      (slice(1024, NPAD), NPAD - 1024)]

        def tail_broadcast(oT, last):
            for nb in range(NPAD // 128):
                p_r = pmr.tile([128, NF], F32, tag="mr")
                nc.tensor.transpose(out=p_r[:], in_=oT[:, 128 * nb:128 * (nb + 1)],
                                    identity=ident_f[:64, :64])
                if last:
                    rows = wp.tile([128, NF], F32, tag="rows_f")
                    nc.vector.tensor_copy(rows[:], p_r[:])
                    nc.sync.dma_start(out_own.ap()[128 * nb:128 * (nb + 1), :], rows[:])
                else:
                    rows = wp.tile([128, NF], BF16, tag="rows_b")
                    nc.vector.tensor_copy(rows[:], p_r[:])
                    nc.sync.dma_start(own_rows.ap()[128 * nb:128 * (nb + 1), :], rows[:])
            if not last:
                nc.gpsimd.collective_compute(
                    "AllGather", mybir.AluOpType.bypass,
                    replica_groups=[ALL_CORES],
                    ins=[own_rows.ap()], outs=[outbuf.ap()])

        # ---- out0 = relu(lin0^T hT + b)
        for sl, n in col_groups:
            p_o = pmsg.tile([64, 512], F32, tag="msg")
            nc.tensor.matmul(p_o[:, :n], lhsT=lin0w_s[:], rhs=hT_s[:, sl],
                             start=True, stop=True, skip_group_check=True)
            nc.scalar.activation(outT[0][:, sl], p_o[:, :n],
                                 mybir.ActivationFunctionType.Relu,
                                 bias=lin0b_s[:])
        tail_broadcast(outT[0], last=False)

        # ---- conv iterations
        for it in range(N_CONV):
            cur = outT[it % 2]
            nxt = outT[(it + 1) % 2]

            agg_tiles = [None] * NBLK
            xg = cp.tile([128, nchunk * NF], BF16, tag=f"xg{it % 2}")
            for ch in range(nchunk):
                nc.gpsimd.indirect_dma_start(
                    out=xg[:, NF * ch:NF * (ch + 1)], out_offset=None,
                    in_=outbuf.ap(),
                    in_offset=IndirectOffsetOnAxis(
                        ap=srcrow_s[:, ch:ch + 1], axis=0))

            for t in range(ntile):
                esl = slice(512 * t, 512 * (t + 1))
                # x^T tile, stacked to 128 partitions
                p_xt = pxt.tile([64, 512], BF16, tag="xt")
                for c4 in range(4):
                    ch = 4 * t + c4
                    nc.tensor.transpose(
                        out=p_xt[:, 128 * c4:128 * (c4 + 1)],
                        in_=xg[:, NF * ch:NF * (ch + 1)],
                        identity=ident_bf[:])
                xts = wp.tile([128, 512], BF16, tag="xts")
                nc.vector.tensor_copy(xts[:64, :], p_xt[:])
                nc.sync.dma_start(xts[64:, :], xts[:64, :])

                p_msg = pmsg.tile([64, 512], F32, tag="msg")
                nc.tensor.matmul(p_msg[:], lhsT=b2_s[:], rhs=xts[:64, :],
                                 start=True, stop=False, skip_group_check=True)
                for kc in range(32):
                    p_b = pb.tile([128, 512], F32, tag="b")
                    nc.tensor.matmul(p_b[:], lhsT=sel_s[:, 128 * kc:128 * (kc + 1)],
                                     rhs=zT_s[:, esl], start=True, stop=True,
                                     skip_group_check=True)
                    bsb = wp.tile([128, 512], BF16, tag="bsb")
                    if kc % 4 < 3:
                        nc.scalar.activation(bsb[:], p_b[:],
                                             mybir.ActivationFunctionType.Copy)
                    else:
                        nc.vector.tensor_copy(bsb[:], p_b[:])
                    u = wp.tile([128, 512], BF16, tag="u")
                    nc.vector.tensor_tensor(out=u[:], in0=bsb[:], in1=xts[:],
                                            op=mybir.AluOpType.mult)
                    nc.tensor.matmul(p_msg[:], lhsT=w2p_s[:, NF * kc:NF * (kc + 1)],
                                     rhs=u[:], start=False, stop=(kc == 31),
                                     skip_group_check=True)

                msgs = wp.tile([64, 512], BF16, tag="msgs")
                nc.vector.tensor_copy(msgs[:], p_msg[:])
                p_mr = pmr.tile([128, 4 * NF], BF16, tag="mr")
                for c4 in range(4):
                    nc.tensor.transpose(
                        out=p_mr[:, NF * c4:NF * (c4 + 1)],
                        in_=msgs[:, 128 * c4:128 * (c4 + 1)],
                        identity=ident_bf[:64, :64])
                msgr = wp.tile([128, 4 * NF], BF16, tag="msgr")
                nc.vector.tensor_copy(msgr[:], p_mr[:])

                for c4 in range(4):
                    ch = 4 * t + c4
                    b = int(blk_of_chunk[ch])
                    if chunk_first[ch]:
                        p_agg = pagg.tile([64, BLK], F32, tag=f"agg{b}")
                        agg_tiles[b] = p_agg
                    p_agg = agg_tiles[b]
                    nc.tensor.matmul(p_agg[:], lhsT=msgr[:, NF * c4:NF * (c4 + 1)],
                                     rhs=sone_s[:, BLK * ch:BLK * (ch + 1)],
                                     start=bool(chunk_first[ch]),
                                     stop=bool(chunk_last[ch]),
                                     skip_group_check=True)
                    if chunk_last[ch]:
                        nc.vector.tensor_copy(aggsb[:, BLK * b:BLK * (b + 1)],
                                              p_agg[:])

            # node update: out' = relu(root^T out + agg + b)
            for g, (sl, n) in enumerate(col_groups):
                p_h2 = pmsg.tile([64, 512], F32, tag="msg")
                nc.tensor.matmul(p_h2[:, :n], lhsT=rootw_s[:], rhs=cur[:, sl],
                                 start=True, stop=True, skip_group_check=True)
                ssum = wp.tile([64, 512], F32, tag="ssum")
                nc.vector.tensor_tensor(
                    out=ssum[:, :n], in0=p_h2[:, :n],
                    in1=aggsb[:, BLK * g:BLK * g + n], op=mybir.AluOpType.add)
                nc.scalar.activation(nxt[:, sl], ssum[:, :n],
                                     mybir.ActivationFunctionType.Relu,
                                     bias=convb_s[:])
            tail_broadcast(nxt, last=(it == N_CONV - 1))

    nc.compile()
    return nc


_CACHE = {}


def _get_nc(meta):
    key = (meta["epad"], tuple(meta["blk_of_chunk"].tolist()))
    if key not in _CACHE:
        _CACHE[key] = _build(meta)
    return _CACHE[key]


def _in_maps(meta, per_core, w):
    maps = []
    for m in range(NCORES):
        d = {
            "srcrow": per_core["srcrow"][m],
            "s_onehot": per_core["s_onehot"][m],
            "eaT": per_core["eaT"][m],
            "hT_own": per_core["hT_own"][m],
        }
        for k in ("lin0_w", "lin0_b", "short_w", "short_b", "nn1_w", "nn1_b",
                  "root_w", "conv_b", "w2p", "b2", "sel_all"):
            d[k] = w[k]
        maps.append(d)
    return maps


def _run(inputs, trace=False):
    meta, per_core, w = _prep(inputs)
    nc = _get_nc(meta)
    res = run_bass_kernel_spmd(nc, _in_maps(meta, per_core, w), ALL_CORES,
                               trace=trace)
    out = np.concatenate(
        [res.results[m]["out_own"][:NPC] for m in range(NCORES)], axis=0)
    return out.astype(np.float32), res


def kernel(**inputs):
    out, _ = _run(inputs, trace=False)
    return out
